# revision 1
# baseline (speedup 1.0000x reference)
"""Causal self-attention (B=4, T=2048, C=1024, H=16, D=64) on 8 trn2 cores.

Sharding: zero-collective. Core = (batch b, parity p): b = core//2, p = core%2.
Each core handles one batch and 4 interleaved 256-query chunks chosen so the
causal attention work is balanced: parity 0 -> chunks [0,7,2,5], parity 1 ->
[1,6,3,4] (of 8 chunks). Every core computes K/V projections for its full
batch (duplicated across the 2 cores of a batch), attention for its queries,
and the output projection rows for its queries. The SPMD program is identical
across cores; all per-core differences enter through DRAM inputs (xTq slices,
additive masks, output scatter done on host).

Device pipeline per core (matmuls in f32r: ~2e-4 rel accuracy at 1 cyc/row
for free dims >= 256; AV matmul in bf16; one shared PSUM pool across stages):
  A) K^T [1024hd, 2048] and V_aug [2048, 8pair, 130] from streamed x^T.
  B) Q^T [1024hd, 1024 local q] from streamed x^T_q.
  C) Flash-style attention per head-pair/slot/key-tile in the transposed
     (key-partition) domain: S^T = K^T.T@Q^T -> +mask -> exp (ACT) ->
     y^T += V_aug.T @ P^T with a fused ones-column giving row sums l.
     Per pair: gather the 8 l-rows via DRAM, one batched DVE reciprocal,
     DMA partition-broadcast of 1/l, scale y^T, store to DRAM.
  D) out = y^T.T @ Wp^T.
"""

import sys
from contextlib import ExitStack

sys.path.insert(0, "/opt/trn_rl_repo")

import numpy as np

import concourse.bass as bass
import concourse.bacc as bacc
import concourse.tile as tile
from concourse import mybir
from concourse.bass_utils import run_bass_kernel_spmd

F32 = mybir.dt.float32
F32R = mybir.dt.float32r
BF16 = mybir.dt.bfloat16

B, T, C, H, D = 4, 2048, 1024, 16, 64
P = 128
NPAIR = H // 2          # 8 head pairs; pair p = heads (2p, 2p+1)
CSUB = C // P           # 8 contraction subtiles
TQL = T // 2            # 1024 local queries per core
NSLOT, QCH = 4, 256     # 4 slots x 256 queries
NJT = T // P            # 16 key tiles of 128
BOUNDS = [4, 8, 12, 16]  # j-tiles processed per slot (uniform across cores)
CHUNKS = [[0, 2, 5, 7], [1, 3, 4, 6]]  # global 256-query chunk per slot
SCALE = 1.0 / 8.0       # 1/sqrt(D)
NEG = -1.0e30

_CACHE = {}
DEBUG = False


def _r(ap):
    return ap.bitcast(F32R)


def build_nc():
    nc = bacc.Bacc("TRN2", target_bir_lowering=False, debug=False)

    xT = nc.dram_tensor("xT", [C, T], F32, kind="ExternalInput")
    xTq = nc.dram_tensor("xTq", [C, TQL], F32, kind="ExternalInput")
    wqT = nc.dram_tensor("wqT", [C, C], F32, kind="ExternalInput")
    wkT = nc.dram_tensor("wkT", [C, C], F32, kind="ExternalInput")
    wvT = nc.dram_tensor("wvT", [C, C], F32, kind="ExternalInput")
    wpT = nc.dram_tensor("wpT", [C, C], F32, kind="ExternalInput")
    # additive causal mask for the last 4 j-tiles of each slot:
    # [j_local 128, slot, rel_jt 4, q_local 256]
    maskd = nc.dram_tensor("mask", [P, NSLOT, 4, QCH], BF16, kind="ExternalInput")
    out = nc.dram_tensor("out", [TQL, C], F32, kind="ExternalOutput")
    if DEBUG:
        dbg_kt = nc.dram_tensor("dbg_kt", [P, NPAIR, T], F32, kind="ExternalOutput")
        dbg_qt = nc.dram_tensor("dbg_qt", [P, NPAIR, TQL], F32, kind="ExternalOutput")
        dbg_vaug = nc.dram_tensor("dbg_vaug", [P, NJT, NPAIR, 130], BF16, kind="ExternalOutput")
        dbg_yT = nc.dram_tensor("dbg_yT", [C, TQL], F32, kind="ExternalOutput")
        dbg_pt = nc.dram_tensor("dbg_pt", [P, 2, 4, QCH], BF16, kind="ExternalOutput")
        dbg_ytu = nc.dram_tensor("dbg_ytu", [65, 2, QCH], F32, kind="ExternalOutput")
        dbg_rb = nc.dram_tensor("dbg_rb", [64, 2, QCH], F32, kind="ExternalOutput")

    with tile.TileContext(nc) as tc:
        with (
            tc.tile_pool(name="dram", bufs=1, space="DRAM") as dramp,
            tc.tile_pool(name="res", bufs=1) as res,
            tc.tile_pool(name="pmm", bufs=3, space="PSUM") as pmm,
        ):
            def mmtile(nm):
                t = pmm.tile([P, 4, QCH], F32, tag="mm", name=nm)
                return t
            yTds = [dramp.tile([C, QCH], F32, name=f"yTd{i}")
                    for i in range(NSLOT)]
            lscr = dramp.tile([NPAIR, 8, QCH], F32)
            rscr = dramp.tile([NPAIR, 8, QCH], F32)
            # Long-lived SBUF residents (stages A-C).
            kts = [res.tile([P, NPAIR, QCH], F32R, name=f"kt{i}")
                   for i in range(T // QCH)]           # K^T, 64KB/part total
            vaugs = [res.tile([P, 2, NPAIR, 130], BF16, name=f"va{i}")
                     for i in range(T // QCH)]         # V+ones, 33KB/part total

            # ones columns of vaug (written once; V copies never touch them)
            vviews = [va.rearrange("a b c (s d) -> a b c s d", s=2)
                      for va in vaugs]
            for vv in vviews:
                nc.vector.memset(vv[:, :, :, :, 64:65], 1.0)

            # warm up the ACT function tables (Exp + Copy) at t~0 so the
            # table DMA overlaps the initial weight loads instead of
            # stalling the first attention exp / stage-D copy
            warm = res.tile([1, 2], F32, name="warm")
            nc.vector.memset(warm, 1.0)
            warm2 = res.tile([1, 2], F32, name="warm2")
            nc.scalar.activation(
                out=warm2, in_=warm,
                func=mybir.ActivationFunctionType.Exp, scale=1.0)
            nc.scalar.copy(out=warm, in_=warm2)

            # ---- Stage A: K^T and V_aug over full T ----
            with (
                tc.tile_pool(name="wkv", bufs=1) as wkvp,
                tc.tile_pool(name="xa", bufs=3) as xap,
            ):
                wk = wkvp.tile([P, CSUB, C], F32R)
                wv = wkvp.tile([P, CSUB, C], F32R)
                for cs in range(CSUB):
                    nc.sync.dma_start(out=wk[:, cs, :], in_=_r(wkT[cs * P:(cs + 1) * P, :]))
                    nc.sync.dma_start(out=wv[:, cs, :], in_=_r(wvT[cs * P:(cs + 1) * P, :]))

                for tch in range(T // QCH):  # 8 chunks of 256 keys
                    xt = xap.tile([P, CSUB, QCH], F32R, tag="xt")
                    for cs in range(CSUB):
                        nc.sync.dma_start(
                            out=xt[:, cs, :],
                            in_=_r(xT[cs * P:(cs + 1) * P, tch * QCH:(tch + 1) * QCH]),
                        )
                    # K^T blocks
                    for p in range(NPAIR):
                        pk = mmtile("pk")[:, 0, :]
                        for cs in range(CSUB):
                            nc.tensor.matmul(
                                pk, wk[:, cs, p * P:(p + 1) * P], xt[:, cs, :],
                                start=(cs == 0), stop=(cs == CSUB - 1),
                            )
                        nc.vector.tensor_copy(out=kts[tch][:, p, :], in_=pk)
                    # V rows (two 128-row subtiles per chunk, two 512 col chunks)
                    for ts in range(2):
                        tidx = tch * 2 + ts
                        for och in range(2):
                            pv = mmtile("pv").rearrange("a b c -> a (b c)")[:, 0:512]
                            for cs in range(CSUB):
                                nc.tensor.matmul(
                                    pv,
                                    xt[:, cs, ts * P:(ts + 1) * P],
                                    wv[:, cs, och * 512:(och + 1) * 512],
                                    start=(cs == 0), stop=(cs == CSUB - 1),
                                )
                            dst = vviews[tch][:, ts, och * 4:(och + 1) * 4, :, 0:64]
                            nc.vector.tensor_copy(
                                out=dst,
                                in_=pv.rearrange("a (b s d) -> a b s d", b=4, s=2),
                            )

            # ---- Stage B: Q^T over local queries ----
            es2 = ExitStack()
            res2 = es2.enter_context(tc.tile_pool(name="res2", bufs=1))
            qts = [res2.tile([P, NPAIR, QCH], F32R, name=f"qt{i}")
                   for i in range(NSLOT)]              # Q^T, 32KB/part total
            with (
                tc.tile_pool(name="wq", bufs=1) as wqp,
                tc.tile_pool(name="xb", bufs=3) as xbp,
            ):
                wq = wqp.tile([P, CSUB, C], F32R)
                for cs in range(CSUB):
                    nc.sync.dma_start(out=wq[:, cs, :], in_=_r(wqT[cs * P:(cs + 1) * P, :]))
                for s in range(NSLOT):
                    xq = xbp.tile([P, CSUB, QCH], F32R, tag="xq")
                    for cs in range(CSUB):
                        nc.sync.dma_start(
                            out=xq[:, cs, :],
                            in_=_r(xTq[cs * P:(cs + 1) * P, s * QCH:(s + 1) * QCH]),
                        )
                    for p in range(NPAIR):
                        pq = mmtile("pq")[:, 0, :]
                        for cs in range(CSUB):
                            nc.tensor.matmul(
                                pq, wq[:, cs, p * P:(p + 1) * P], xq[:, cs, :],
                                start=(cs == 0), stop=(cs == CSUB - 1),
                            )
                        nc.vector.tensor_copy(out=qts[s][:, p, :], in_=pq)

            if DEBUG:
                for i in range(T // QCH):
                    nc.sync.dma_start(
                        out=dbg_kt[:, :, i * QCH:(i + 1) * QCH],
                        in_=kts[i].bitcast(F32))
                    nc.sync.dma_start(
                        out=dbg_vaug[:, i * 2:(i + 1) * 2, :, :], in_=vaugs[i])
                for i in range(NSLOT):
                    nc.sync.dma_start(
                        out=dbg_qt[:, :, i * QCH:(i + 1) * QCH],
                        in_=qts[i].bitcast(F32))

            # ---- Stage C: attention ----
            es3 = ExitStack()
            res3 = es3.enter_context(tc.tile_pool(name="res3", bufs=1))
            mask = res3.tile([P, NSLOT, 4, QCH], BF16)
            nc.sync.dma_start(out=mask, in_=maskd[:, :, :, :])

            with (
                tc.tile_pool(name="att", bufs=6) as att,
                tc.tile_pool(name="yus", bufs=2) as yusp,
            ):
                for p in range(NPAIR):
                    yu = yusp.tile([65, 8, QCH], F32, tag="yu")
                    for s in range(NSLOT):
                        nj = BOUNDS[s]
                        ngrp = nj // 4
                        yth = [pmm.tile([65, QCH], F32, tag=f"yt{hi}", name=f"yt{hi}", bufs=1) for hi in range(2)]
                        for hi in range(2):
                            for g in range(ngrp):
                                h0 = hi * 64
                                st4 = mmtile("st4")
                                for i in range(4):
                                    jt = g * 4 + i
                                    nc.tensor.matmul(
                                        st4[:, i, :],
                                        kts[jt // 2][h0:h0 + 64, p, (jt % 2) * P:(jt % 2 + 1) * P],
                                        qts[s][h0:h0 + 64, p, :],
                                        start=True, stop=True,
                                    )
                                pt4 = att.tile([P, 4, QCH], BF16, tag="pt")
                                nc.scalar.activation(
                                    out=pt4, in_=st4,
                                    func=mybir.ActivationFunctionType.Exp,
                                    scale=SCALE,
                                )
                                if g == ngrp - 1:
                                    nc.vector.tensor_mul(pt4, pt4, mask[:, s, :, :])
                                if DEBUG and p == 0 and s == 0:
                                    for i in range(4):
                                        nc.sync.dma_start(
                                            out=dbg_pt[:, hi, g * 4 + i, :],
                                            in_=pt4[:, i, :])
                                for i in range(4):
                                    jt = g * 4 + i
                                    nc.tensor.matmul(
                                        yth[hi],
                                        vaugs[jt // 2][:, jt % 2, p, hi * 65:(hi + 1) * 65],
                                        pt4[:, i, :],
                                        start=(jt == 0), stop=(jt == nj - 1),
                                    )
                        # stash unnormalized y + l row; free psum
                        for hi in range(2):
                            r = s * 2 + hi
                            nc.vector.tensor_copy(out=yu[:, r, :], in_=yth[hi])
                            if DEBUG and p == 0 and s == 0:
                                nc.sync.dma_start(out=dbg_ytu[:, hi, :], in_=yu[:, r, :])
                    # batched normalization for the whole pair
                    for r in range(8):
                        nc.sync.dma_start(out=lscr[p, r], in_=yu[64:65, r, :])
                    lsb = att.tile([8, QCH], F32, tag="lsb")
                    nc.sync.dma_start(out=lsb, in_=lscr[p, :, :])
                    rinv = att.tile([8, QCH], F32, tag="rinv")
                    nc.vector.reciprocal(out=rinv, in_=lsb)
                    nc.sync.dma_start(out=rscr[p, :, :], in_=rinv)
                    for s in range(NSLOT):
                        for hi in range(2):
                            r = s * 2 + hi
                            rd = rscr[p, r]
                            rb = att.tile([64, QCH], F32, tag="rb")
                            nc.sync.dma_start(
                                out=rb,
                                in_=bass.AP(
                                    tensor=rd.tensor, offset=rd.offset,
                                    ap=[[0, 64]] + [list(a) for a in rd.ap],
                                ),
                            )
                            if DEBUG and p == 0 and s == 0:
                                nc.sync.dma_start(out=dbg_rb[:, hi, :], in_=rb)
                            ysb = att.tile([64, QCH], F32, tag="ysb")
                            nc.vector.tensor_mul(ysb, yu[0:64, r, :], rb)
                            nc.sync.dma_start(
                                out=yTds[s][p * P + hi * 64:p * P + (hi + 1) * 64, :],
                                in_=ysb,
                            )

            if DEBUG:
                for i in range(NSLOT):
                    nc.sync.dma_start(
                        out=dbg_yT[:, i * QCH:(i + 1) * QCH], in_=yTds[i][:, :])

            es3.close()
            es2.close()

            # ---- Stage D: output projection ----
            with (
                tc.tile_pool(name="work", bufs=4) as work,
                tc.tile_pool(name="wp", bufs=1) as wpp2,
                tc.tile_pool(name="yd", bufs=3) as ydp,
            ):
                wp = wpp2.tile([P, CSUB, C], F32R)
                for cs in range(CSUB):
                    nc.sync.dma_start(out=wp[:, cs, :], in_=_r(wpT[cs * P:(cs + 1) * P, :]))
                for s2 in range(NSLOT):
                    ylw = ydp.tile([P, CSUB, QCH], F32R, tag="yl")
                    for cb in range(CSUB):
                        nc.sync.dma_start(
                            out=ylw[:, cb, :],
                            in_=_r(yTds[s2][cb * P:(cb + 1) * P, :]),
                        )
                    for qh in range(2):
                        qs = s2 * 2 + qh
                        for och in range(2):
                            po = mmtile("po").rearrange("a b c -> a (b c)")[:, 0:512]
                            for cb in range(CSUB):
                                nc.tensor.matmul(
                                    po,
                                    ylw[:, cb, qh * P:(qh + 1) * P],
                                    wp[:, cb, och * 512:(och + 1) * 512],
                                    start=(cb == 0), stop=(cb == CSUB - 1),
                                )
                            osb = work.tile([P, 512], F32, tag="osb")
                            if (s2 * 4 + qh * 2 + och) % 2 == 0:
                                nc.scalar.copy(out=osb, in_=po)
                            else:
                                nc.vector.tensor_copy(out=osb, in_=po)
                            nc.sync.dma_start(
                                out=out[qs * P:(qs + 1) * P, och * 512:(och + 1) * 512],
                                in_=osb,
                            )

    nc.compile()
    return nc


def _make_mask(parity: int) -> np.ndarray:
    import ml_dtypes
    m = np.zeros((P, NSLOT, 4, QCH), dtype=np.float32)
    for s in range(NSLOT):
        c = CHUNKS[parity][s]
        for i in range(4):
            jt = BOUNDS[s] - 4 + i
            jg = jt * P + np.arange(P)[:, None]          # key index
            qg = c * QCH + np.arange(QCH)[None, :]       # query index
            m[:, s, i, :] = np.where(jg <= qg, 1.0, 0.0)
    return m.astype(ml_dtypes.bfloat16)


def kernel(x, Wq, bq, Wk, bk, Wv, bv, Wp, bp):
    x = np.asarray(x, dtype=np.float32)
    assert x.shape == (B, T, C)
    for b_ in (bq, bk, bv, bp):
        assert not np.any(np.asarray(b_)), "nonzero biases unsupported"

    if "nc" not in _CACHE:
        _CACHE["nc"] = build_nc()
    nc = _CACHE["nc"]

    wqT = np.ascontiguousarray(np.asarray(Wq, np.float32).T)
    wkT = np.ascontiguousarray(np.asarray(Wk, np.float32).T)
    wvT = np.ascontiguousarray(np.asarray(Wv, np.float32).T)
    wpT = np.ascontiguousarray(np.asarray(Wp, np.float32).T)
    masks = [_make_mask(0), _make_mask(1)]

    in_maps = []
    for core in range(8):
        b, par = core // 2, core % 2
        xT = np.ascontiguousarray(x[b].T)
        xTq = np.ascontiguousarray(
            np.concatenate(
                [xT[:, c * QCH:(c + 1) * QCH] for c in CHUNKS[par]], axis=1)
        )
        in_maps.append(
            dict(xT=xT, xTq=xTq, wqT=wqT, wkT=wkT, wvT=wvT, wpT=wpT,
                 mask=masks[par])
        )

    _CACHE["last_in_maps"] = in_maps
    try:
        res = run_bass_kernel_spmd(nc, in_maps, core_ids=list(range(8)))
    except Exception:
        # the axon device occasionally reports NRT_EXEC_UNIT_UNRECOVERABLE;
        # resetting the PJRT backend and retrying once recovers it
        import jax
        try:
            jax.clear_caches()
            jax.extend.backend.clear_backends()
        except Exception:
            pass
        res = run_bass_kernel_spmd(nc, in_maps, core_ids=list(range(8)))

    out = np.empty((B, T, C), dtype=np.float32)
    for core in range(8):
        ol = res.results[core]["out"]
        b, par = core // 2, core % 2
        for s, c in enumerate(CHUNKS[par]):
            out[b, c * QCH:(c + 1) * QCH] = ol[s * QCH:(s + 1) * QCH]
    return out

